# revision 1
# baseline (speedup 1.0000x reference)
"""DeepseekV2 MLA attention fusion on 8 Trainium2 NeuronCores.

Strategy (tensor-parallel over heads + T-sharded low-rank projections):
  - Host: shard/transpose/cast inputs. Everything on-device is kept in a
    "features-on-partitions, T-on-free" (transposed) layout so no on-device
    transposes are ever needed.
  - Stage A (per core, T-shard of 256 rows): a-projections (q_a, kv_a) with
    weights stationary -> rmsnorm (partition-dim reduction done with an
    all-ones matmul on the PE) -> rope on k_pe -> bf16 -> AllGather #1.
  - Stage B: q/kv up-projections for the core's 4 heads, rope on q_pe.
  - Attention: scores^T[k,q] on PE (causal blocks only, variable width),
    exp on ScalarE with the softmax scale folded in (no max subtraction --
    scores are O(10) for this distribution so exp is safe in fp32),
    lower-triangle mask on diagonal blocks, P@V plus an all-ones matmul for
    the softmax row sums accumulated in PSUM, normalization at the end.
  - AllGather #2 of attn^T (bf16), then each core computes a 512-wide HID
    slice of the output projection; host concatenates slices.
"""

import numpy as np
import ml_dtypes

import concourse.bass as bass
import concourse.mybir as mybir
import concourse.tile as tile
from concourse import bacc
from concourse.masks import make_upper_triangular

T = 2048
HID = 4096
NH = 32
DN = 128
DR = 64
DV = 128
QLR = 1536
KVLR = 512
EPS = 1e-6
THETA = 10000.0
SCALE = float((DN + DR) ** -0.5)

NCORES = 8
HL = NH // NCORES          # 4 heads per core
TL = T // NCORES           # 256 sequence rows per core
FQ = QLR // 128            # 12 qlr chunks
FKV = KVLR // 128          # 4 kvlr chunks
KH = HID // 128            # 32 hid chunks
FA = QLR + KVLR + DR       # 2112 a-proj output features
MA = 17                    # a-proj M tiles (2176 = 17*128, zero padded)
NT = T // 128              # 16 T tiles
NCH = T // 512             # 4 column chunks of 512
OSL = HID // NCORES        # 512-wide o-proj output slice per core

BF = mybir.dt.bfloat16
F32 = mybir.dt.float32
NPBF = ml_dtypes.bfloat16


def build_module(n_rep: int = 1, upto: str = "D", fake_coll: bool = False):
    """Build the Bass module (same program for every core).

    fake_coll=True replaces the collectives with local DMA copies of
    equivalent size so the module is collective-free (for TimelineSim).
    """
    nc = bacc.Bacc("TRN2", target_bir_lowering=False, debug=False,
                   num_devices=1 if fake_coll else NCORES)

    hsT = nc.dram_tensor("hsT", [HID, TL], BF, kind="ExternalInput")
    wa = nc.dram_tensor("wa", [MA, KH, 128, 128], BF, kind="ExternalInput")
    wqb = nc.dram_tensor("wqb", [6, FQ, 128, 128], BF, kind="ExternalInput")
    wkn = nc.dram_tensor("wkn", [HL, FKV, 128, 128], BF, kind="ExternalInput")
    wv = nc.dram_tensor("wv", [FKV, 128, HL * DV], BF, kind="ExternalInput")
    wo = nc.dram_tensor("wo", [NH * DV // 128, 128, OSL], BF,
                        kind="ExternalInput")
    cosq = nc.dram_tensor("cosq", [128, T], F32, kind="ExternalInput")
    sgnsinq = nc.dram_tensor("sgnsinq", [128, T], F32, kind="ExternalInput")
    cosk = nc.dram_tensor("cosk", [64, TL], F32, kind="ExternalInput")
    sgnsink = nc.dram_tensor("sgnsink", [64, TL], F32, kind="ExternalInput")
    out_o = nc.dram_tensor("out_o", [T, OSL], F32, kind="ExternalOutput")

    with tile.TileContext(nc) as tc:
        with tc.tile_pool(name="const", bufs=1) as const_pool:
            ones_f32 = const_pool.tile([128, 128], F32)
            nc.vector.memset(ones_f32, 1.0)
            ones_bf = const_pool.tile([128, 128], BF)
            nc.vector.memset(ones_bf, 1.0)
            trimask = const_pool.tile([128, 128], BF)
            make_upper_triangular(nc, trimask[:], val=1.0, diag=True)
            eps_sb = const_pool.tile([128, 1], F32)
            nc.vector.memset(eps_sb, EPS)
            cosq_sb = const_pool.tile([128, T], F32)
            nc.sync.dma_start(out=cosq_sb, in_=cosq.ap())
            sgnsinq_sb = const_pool.tile([128, T], F32)
            nc.sync.dma_start(out=sgnsinq_sb, in_=sgnsinq.ap())
            cosk_sb = const_pool.tile([64, TL], F32)
            nc.sync.dma_start(out=cosk_sb, in_=cosk.ap())
            sgnsink_sb = const_pool.tile([64, TL], F32)
            nc.sync.dma_start(out=sgnsink_sb, in_=sgnsink.ap())

            for _rep in range(n_rep):
                _body(nc, tc, hsT, wa, wqb, wkn, wv, wo, out_o,
                      ones_f32, ones_bf, trimask, eps_sb,
                      cosq_sb, sgnsinq_sb, cosk_sb, sgnsink_sb, upto,
                      fake_coll)

    nc.compile()
    return nc


def _body(nc, tc, hsT, wa, wqb, wkn, wv, wo, out_o,
          ones_f32, ones_bf, trimask, eps_sb,
          cosq_sb, sgnsinq_sb, cosk_sb, sgnsink_sb, upto="D",
          fake_coll=False):
    from contextlib import ExitStack

    def all_gather(ag_in_ap, ag_out, nrows):
        # Collectives >= ~1MB/rank fall into a pathologically slow RDH
        # path in this environment -- keep every piece under the mesh
        # cutoff (callers pre-split).
        if fake_coll:
            for r in range(NCORES):
                nc.sync.dma_start(
                    out=ag_out[nrows * r:nrows * (r + 1), :], in_=ag_in_ap)
        else:
            nc.gpsimd.collective_compute(
                "AllGather", mybir.AluOpType.bypass,
                ins=[ag_in_ap.opt()], outs=[ag_out.opt()],
                replica_groups=[list(range(NCORES))])

    def dbg_drain(pool, src_ap, n):
        """Cast a [128, n] tile to f32 and write it to out_o[0:128, 0:n]."""
        dbg = pool.tile([128, n], F32, tag="dbg", name="dbg")
        nc.vector.tensor_copy(out=dbg[:], in_=src_ap)
        nc.sync.dma_start(out=out_o.ap()[0:128, 0:n], in_=dbg[:])

    with ExitStack() as phases:
        dram = phases.enter_context(
            tc.tile_pool(name="dram", bufs=1, space="DRAM"))
        shared = "Local" if fake_coll else "Shared"
        # AG1 split: rows 0:1024 and 1024:2112 (512/544 KB per rank).
        # Each split collective gets its own whole-tensor input.
        ag1i0 = dram.tile([1024, TL], BF)
        ag1i1 = dram.tile([FA - 1024, TL], BF)
        ag1o0 = dram.tile([1024 * NCORES, TL], BF, addr_space=shared)
        ag1o1 = dram.tile([(FA - 1024) * NCORES, TL], BF, addr_space=shared)
        # AG2 split: one collective per local head (512 KB per rank)
        ag2i = [dram.tile([DV, T], BF, name=f"ag2i{h}") for h in range(HL)]
        ag2o = [dram.tile([DV * NCORES, T], BF, addr_space=shared,
                          name=f"ag2o{h}") for h in range(HL)]

        def ag1_in_slice(row0, nrows):
            # feature rows [row0, row0+nrows) of the stage-A output --
            # returns the AP inside the right split-input tile
            if row0 < 1024:
                assert row0 + nrows <= 1024
                return ag1i0[row0:row0 + nrows, :]
            return ag1i1[row0 - 1024:row0 - 1024 + nrows, :]

        # ---------------- Phase A: a-projections + rmsnorm + k_pe rope ----
        with ExitStack() as pa:
            hs_pool = pa.enter_context(tc.tile_pool(name="hsT", bufs=1))
            wa_pool = pa.enter_context(tc.tile_pool(name="wa", bufs=6))
            psA = pa.enter_context(
                tc.tile_pool(name="psA", bufs=2, space="PSUM"))
            psR = pa.enter_context(
                tc.tile_pool(name="psR", bufs=1, space="PSUM"))
            rawA = pa.enter_context(tc.tile_pool(name="rawA", bufs=17))
            sqA = pa.enter_context(tc.tile_pool(name="sqA", bufs=3))
            nrmA = pa.enter_context(tc.tile_pool(name="nrmA", bufs=4))
            ropeA = pa.enter_context(tc.tile_pool(name="ropeA", bufs=1))

            hsT_sb = hs_pool.tile([128, KH, TL], BF)
            nc.sync.dma_start(
                out=hsT_sb, in_=hsT.ap().rearrange("(k p) t -> p k t", p=128))

            rs_q = psR.tile([128, TL], F32)
            rs_kv = psR.tile([128, TL], F32)
            raws = []
            for m in range(MA):
                ps = psA.tile([128, TL], F32)
                for k in range(KH):
                    wt = wa_pool.tile([128, 128], BF)
                    nc.sync.dma_start(out=wt, in_=wa.ap()[m, k])
                    nc.tensor.matmul(ps[:], wt[:], hsT_sb[:, k, :],
                                     start=(k == 0), stop=(k == KH - 1))
                raw = rawA.tile([128, TL], F32)
                nc.vector.tensor_copy(out=raw, in_=ps[:])
                raws.append(raw)
                if m < FQ + FKV:
                    sq = sqA.tile([128, TL], F32)
                    nc.scalar.activation(sq[:], ps[:],
                                         mybir.ActivationFunctionType.Square)
                    if m < FQ:
                        nc.tensor.matmul(rs_q[:], ones_f32[:], sq[:],
                                         start=(m == 0), stop=(m == FQ - 1))
                    else:
                        nc.tensor.matmul(rs_kv[:], ones_f32[:], sq[:],
                                         start=(m == FQ),
                                         stop=(m == FQ + FKV - 1))

            # rsqrt(mean + eps), broadcast across partitions already
            rq = sqA.tile([128, TL], F32, tag="rq")
            nc.scalar.activation(rq[:], rs_q[:],
                                 mybir.ActivationFunctionType.Sqrt,
                                 bias=eps_sb[:], scale=1.0 / QLR)
            nc.vector.reciprocal(rq[:], rq[:])
            rkv = sqA.tile([128, TL], F32, tag="rkv")
            nc.scalar.activation(rkv[:], rs_kv[:],
                                 mybir.ActivationFunctionType.Sqrt,
                                 bias=eps_sb[:], scale=1.0 / KVLR)
            nc.vector.reciprocal(rkv[:], rkv[:])

            for m in range(FQ + FKV):
                nrm = nrmA.tile([128, TL], BF)
                nc.vector.tensor_mul(nrm[:], raws[m][:],
                                     rq[:] if m < FQ else rkv[:])
                nc.sync.dma_start(out=ag1_in_slice(m * 128, 128),
                                  in_=nrm[:])

            # k_pe rope (raws[16] rows 0:64; rows 0:32 = x1, 32:64 = x2).
            # Partition moves must go through DMA (walrus enforces equal
            # start partitions on DVE operands).
            kpe_raw = raws[16]
            kswap = ropeA.tile([64, TL], F32, tag="kswap")
            nc.sync.dma_start(out=kswap[0:32, :], in_=kpe_raw[32:64, :])
            nc.sync.dma_start(out=kswap[32:64, :], in_=kpe_raw[0:32, :])
            ku = ropeA.tile([64, TL], F32, tag="ku")
            kw = ropeA.tile([64, TL], F32, tag="kw")
            nc.vector.tensor_mul(ku[:], kpe_raw[0:64, :], cosk_sb[:])
            nc.vector.tensor_mul(kw[:], kswap[:], sgnsink_sb[:])
            kpe_out = ropeA.tile([64, TL], BF, tag="kpeo")
            nc.vector.tensor_add(kpe_out[:], ku[:], kw[:])
            nc.sync.dma_start(out=ag1_in_slice(2048, 64), in_=kpe_out[:])

            if upto == "A":
                dbg_drain(ropeA, raws[0][:], TL)
                return

        # ---------------- AllGather #1 -----------------------------------
        all_gather(ag1i0[:], ag1o0, 1024)
        all_gather(ag1i1[:], ag1o1, FA - 1024)

        ag1a0, ag1a1 = ag1o0[:], ag1o1[:]
        rs0 = 1024 * TL   # rank stride, first split
        rs1 = (FA - 1024) * TL  # rank stride, second split

        if upto == "AG1":
            with tc.tile_pool(name="dbgP", bufs=1) as dbgP:
                t = dbgP.tile([128, TL], BF)
                nc.sync.dma_start(out=t, in_=ag1o0[0:128, :])
                dbg_drain(dbgP, t[:], TL)
            return

        # ---------------- Phase B: up-projections + q rope ----------------
        big = phases.enter_context(tc.tile_pool(name="bigB", bufs=1))
        # attention-phase operands (stay alive through phase C)
        qn_sb = [big.tile([128, T], BF, tag=f"qn{h}", name=f"qn{h}")
                 for h in range(HL)]
        rp_sb = [big.tile([64, T], BF, tag=f"rp{i}", name=f"rp{i}")
                 for i in range(HL)]
        kn_sb = [big.tile([128, T], BF, tag=f"kn{h}", name=f"kn{h}")
                 for h in range(HL)]
        kpe_sb = big.tile([64, NCORES, TL], BF, tag="kpe", name="kpe")
        v_sb = [big.tile([128, HL * DV], BF, tag=f"v{j}", name=f"v{j}")
                for j in range(NT)]

        nc.sync.dma_start(
            out=kpe_sb,
            in_=bass.AP(tensor=ag1a1.tensor,
                        offset=ag1a1.offset + 1024 * TL,
                        ap=[[TL, 64], [rs1, NCORES], [1, TL]]))

        with ExitStack() as pb:
            wqb_pool = pb.enter_context(tc.tile_pool(name="wqb", bufs=1))
            kvc_pool = pb.enter_context(tc.tile_pool(name="kvc", bufs=1))
            qcc_pool = pb.enter_context(tc.tile_pool(name="qcc", bufs=2))
            psB = pb.enter_context(
                tc.tile_pool(name="psB", bufs=4, space="PSUM"))
            ropeB = pb.enter_context(tc.tile_pool(name="ropeB", bufs=2))

            wqb_sb = wqb_pool.tile([128, 6, FQ, 128], BF)
            for m in range(6):
                nc.sync.dma_start(
                    out=wqb_sb[:, m, :, :],
                    in_=wqb.ap()[m].rearrange("k p q -> p k q"))
            wkn_sb = wqb_pool.tile([128, HL, FKV, 128], BF, tag="wkn")
            for m in range(HL):
                nc.sync.dma_start(
                    out=wkn_sb[:, m, :, :],
                    in_=wkn.ap()[m].rearrange("k p q -> p k q"))
            wv_sb = wqb_pool.tile([128, FKV, HL * DV], BF, tag="wv")
            nc.sync.dma_start(out=wv_sb,
                              in_=wv.ap().rearrange("k p n -> p k n"))
            kvc_sb = kvc_pool.tile([128, FKV, NCORES, TL], BF)
            for r in range(NCORES):
                nc.sync.dma_start(
                    out=kvc_sb[:, :, r, :],
                    in_=bass.AP(tensor=ag1a1.tensor,
                                offset=ag1a1.offset + r * rs1 + 512 * TL,
                                ap=[[TL, 128], [128 * TL, FKV], [1, TL]]))

            # q up-projection, chunk by chunk over columns
            for c in range(NCH):
                qc_c = qcc_pool.tile([128, FQ, 2, TL], BF)
                for rr in range(2):
                    nc.sync.dma_start(
                        out=qc_c[:, 0:8, rr, :],
                        in_=bass.AP(
                            tensor=ag1a0.tensor,
                            offset=ag1a0.offset + (2 * c + rr) * rs0,
                            ap=[[TL, 128], [128 * TL, 8], [1, TL]]))
                    nc.sync.dma_start(
                        out=qc_c[:, 8:FQ, rr, :],
                        in_=bass.AP(
                            tensor=ag1a1.tensor,
                            offset=ag1a1.offset + (2 * c + rr) * rs1,
                            ap=[[TL, 128], [128 * TL, FQ - 8], [1, TL]]))
                for m in range(6):
                    ps = psB.tile([128, 512], F32, tag="ps")
                    for kc in range(FQ):
                        nc.tensor.matmul(ps[:], wqb_sb[:, m, kc, :],
                                         qc_c[:, kc, :, :],
                                         start=(kc == 0), stop=(kc == FQ - 1))
                    if m < HL:
                        nc.vector.tensor_copy(
                            out=qn_sb[m][:, 512 * c:512 * (c + 1)], in_=ps[:])
                    else:
                        # rope pair tile (two heads of 64 rows each).
                        # All partition moves via DMA; DVE ops aligned.
                        pair = m - HL
                        qraw = ropeB.tile([128, 512], F32, tag="qraw")
                        nc.vector.tensor_copy(out=qraw[:], in_=ps[:])
                        qsw = ropeB.tile([128, 512], F32, tag="qsw")
                        for half in range(4):
                            a, b = 32 * half, 32 * (half + 1)
                            s0 = b if half % 2 == 0 else a - 32
                            nc.sync.dma_start(out=qsw[a:b, :],
                                              in_=qraw[s0:s0 + 32, :])
                        qu = ropeB.tile([128, 512], F32, tag="qu")
                        qw = ropeB.tile([128, 512], F32, tag="qw")
                        nc.vector.tensor_mul(
                            qu[:], qraw[:], cosq_sb[:, 512 * c:512 * (c + 1)])
                        nc.vector.tensor_mul(
                            qw[:], qsw[:],
                            sgnsinq_sb[:, 512 * c:512 * (c + 1)])
                        rope128 = ropeB.tile([128, 512], BF, tag="rope128")
                        nc.vector.tensor_add(rope128[:], qu[:], qw[:])
                        nc.sync.dma_start(
                            out=rp_sb[2 * pair][:, 512 * c:512 * (c + 1)],
                            in_=rope128[0:64, :])
                        nc.sync.dma_start(
                            out=rp_sb[2 * pair + 1][:, 512 * c:512 * (c + 1)],
                            in_=rope128[64:128, :])

                # k_nope for this column chunk
                for m in range(HL):
                    ps = psB.tile([128, 512], F32, tag="ps")
                    for kc in range(FKV):
                        nc.tensor.matmul(
                            ps[:], wkn_sb[:, m, kc, :],
                            kvc_sb[:, kc, 2 * c:2 * c + 2, :],
                            start=(kc == 0), stop=(kc == FKV - 1))
                    nc.vector.tensor_copy(
                        out=kn_sb[m][:, 512 * c:512 * (c + 1)], in_=ps[:])

            # v (natural layout): one T-tile at a time
            for j in range(NT):
                ps = psB.tile([128, 512], F32, tag="ps")
                r, t0 = j // 2, 128 * (j % 2)
                for kc in range(FKV):
                    nc.tensor.matmul(ps[:], kvc_sb[:, kc, r, t0:t0 + 128],
                                     wv_sb[:, kc, :],
                                     start=(kc == 0), stop=(kc == FKV - 1))
                nc.vector.tensor_copy(out=v_sb[j][:], in_=ps[:])

            if upto == "B":
                dbg_drain(ropeB, v_sb[0][:], 512)
                return

        # ---------------- Phase C: attention ------------------------------
        with ExitStack() as pc:
            psSC = pc.enter_context(
                tc.tile_pool(name="psSC", bufs=3, space="PSUM"))
            psAT = pc.enter_context(
                tc.tile_pool(name="psAT", bufs=2, space="PSUM"))
            psSM = pc.enter_context(
                tc.tile_pool(name="psSM", bufs=2, space="PSUM"))
            pP = pc.enter_context(tc.tile_pool(name="pP", bufs=6))
            recP = pc.enter_context(tc.tile_pool(name="recP", bufs=2))
            atP = pc.enter_context(tc.tile_pool(name="atP", bufs=1))
            attn_sb = [atP.tile([128, T], BF, tag=f"at{h}", name=f"at{h}")
                       for h in range(HL)]

            for h in range(HL):
                qpe = rp_sb[h][:]
                for c in range(NCH):
                    attn_ps = psAT.tile([128, 512], F32)
                    sums_ps = psSM.tile([128, 512], F32)
                    jmax = 4 * c + 3
                    for j in range(jmax + 1):
                        off = max(0, 128 * j - 512 * c)
                        sc = psSC.tile([128, 512], F32)
                        nc.tensor.matmul(
                            sc[:, off:], kn_sb[h][:, 128 * j:128 * (j + 1)],
                            qn_sb[h][:, 512 * c + off:512 * (c + 1)],
                            start=True, stop=False)
                        nc.tensor.matmul(
                            sc[:, off:],
                            kpe_sb[:, j // 2, 128 * (j % 2):128 * (j % 2) + 128],
                            qpe[:, 512 * c + off:512 * (c + 1)],
                            start=False, stop=True)
                        p_sb = pP.tile([128, 512], BF)
                        nc.scalar.activation(p_sb[:, off:], sc[:, off:],
                                             mybir.ActivationFunctionType.Exp,
                                             scale=SCALE)
                        if j >= 4 * c:
                            nc.vector.tensor_mul(p_sb[:, off:off + 128],
                                                 p_sb[:, off:off + 128],
                                                 trimask[:])
                        nc.tensor.matmul(attn_ps[:, off:],
                                         v_sb[j][:, DV * h:DV * (h + 1)],
                                         p_sb[:, off:],
                                         start=(j == 0), stop=(j == jmax))
                        nc.tensor.matmul(sums_ps[:, off:], ones_bf[:],
                                         p_sb[:, off:],
                                         start=(j == 0), stop=(j == jmax))
                    rec = recP.tile([128, 512], F32)
                    nc.vector.reciprocal(rec[:], sums_ps[:])
                    nc.vector.tensor_mul(
                        attn_sb[h][:, 512 * c:512 * (c + 1)],
                        attn_ps[:], rec[:])
                    nc.sync.dma_start(
                        out=ag2i[h][:, 512 * c:512 * (c + 1)],
                        in_=attn_sb[h][:, 512 * c:512 * (c + 1)])

            if upto == "C":
                dbg_drain(recP, attn_sb[0][:, 0:512], 512)
                return

        # ---------------- AllGather #2 ------------------------------------
        for h in range(HL):
            all_gather(ag2i[h][:], ag2o[h], DV)

        if upto == "AG2":
            with tc.tile_pool(name="dbgP2", bufs=1) as dbgP2:
                t2 = dbgP2.tile([128, 512], BF)
                nc.sync.dma_start(out=t2, in_=ag2o[0][0:128, 0:512])
                dbg_drain(dbgP2, t2[:], 512)
            return

        # ---------------- Phase D: output projection slice ----------------
        # 8 T-tiles (= 8 PSUM banks) per pass; attn chunks streamed as
        # contiguous [128, 1024] row-blocks of the gathered attn^T.
        with ExitStack() as pd:
            woP = pd.enter_context(tc.tile_pool(name="woP", bufs=1))
            ogP = pd.enter_context(tc.tile_pool(name="ogP", bufs=4))
            psO = pd.enter_context(
                tc.tile_pool(name="psO", bufs=8, space="PSUM"))
            oP = pd.enter_context(tc.tile_pool(name="oP", bufs=4))

            wo_sb = woP.tile([128, NH * DV // 128, OSL], BF)
            nc.sync.dma_start(
                out=wo_sb, in_=wo.ap().rearrange("k p n -> p k n"))

            KO = NH * DV // 128  # 32
            for grp in range(2):
                pss = [psO.tile([128, OSL], F32, tag="pso", name=f"pso{grp}_{t8}")
                       for t8 in range(8)]
                for k in range(KO):
                    og = ogP.tile([128, 1024], BF, tag="og", name="og")
                    nc.sync.dma_start(
                        out=og,
                        in_=ag2o[k % HL][128 * (k // HL):128 * (k // HL + 1),
                                         1024 * grp:1024 * (grp + 1)])
                    for t8 in range(8):
                        nc.tensor.matmul(
                            pss[t8][:], og[:, 128 * t8:128 * (t8 + 1)],
                            wo_sb[:, k, :],
                            start=(k == 0), stop=(k == KO - 1))
                for t8 in range(8):
                    tt = 8 * grp + t8
                    o_sb = oP.tile([128, OSL], F32, tag="osb", name="osb")
                    nc.vector.tensor_copy(out=o_sb, in_=pss[t8][:])
                    nc.sync.dma_start(
                        out=out_o.ap()[128 * tt:128 * (tt + 1), :],
                        in_=o_sb[:])


# ---------------------------------------------------------------------------
# Host side
# ---------------------------------------------------------------------------

_ROPE_PERM = np.concatenate([np.arange(0, DR, 2), np.arange(1, DR, 2)])


def _prepare_inputs(positions, hidden_states, w_qa, w_kva, g_qa, w_qb,
                    g_kva, w_kvb, w_o):
    """Build the 8 per-core input dicts (numpy, host-side layout prep)."""
    positions = np.asarray(positions)
    hs = np.asarray(hidden_states, dtype=np.float32)
    w_qa = np.asarray(w_qa, np.float32)
    w_kva = np.asarray(w_kva, np.float32)
    # rmsnorm(y, g) @ W == rmsnorm_nogain(y) @ (g[:, None] * W)
    w_qb = np.asarray(w_qb, np.float32) * np.asarray(
        g_qa, np.float32)[:, None]
    w_kvb = np.asarray(w_kvb, np.float32) * np.asarray(
        g_kva, np.float32)[:, None]
    w_o = np.asarray(w_o, np.float32)

    # a-projection weights: [w_qa | w_kva_c | w_kva_pe(perm)] zero-padded
    wa_full = np.zeros((HID, MA * 128), np.float32)
    wa_full[:, :QLR] = w_qa
    wa_full[:, QLR:QLR + KVLR] = w_kva[:, :KVLR]
    wa_full[:, QLR + KVLR:QLR + KVLR + DR] = w_kva[:, KVLR:][:, _ROPE_PERM]
    wa_t = np.ascontiguousarray(
        wa_full.reshape(KH, 128, MA, 128).transpose(2, 0, 1, 3)
    ).astype(NPBF)  # [MA, KH, 128, 128]

    # rope tables
    inv_freq = (1.0 / (THETA ** (np.arange(0, DR, 2, dtype=np.float32) / DR))
                ).astype(np.float32)
    f = positions.astype(np.float32)[:, None] * inv_freq[None, :]  # [T, 32]
    cos = np.cos(f).astype(np.float32).T  # [32, T]
    sin = np.sin(f).astype(np.float32).T
    cosq128 = np.tile(cos, (4, 1))
    sgnsinq128 = np.concatenate([-sin, sin, -sin, sin], axis=0)

    w_qb3 = w_qb.reshape(QLR, NH, DN + DR)
    w_kvb3 = w_kvb.reshape(KVLR, NH, DN + DV)
    w_o2 = w_o  # [NH*DV, HID]

    in_maps = []
    for d in range(NCORES):
        heads = range(HL * d, HL * (d + 1))
        hsT_d = np.ascontiguousarray(
            hs[TL * d:TL * (d + 1), :].T).astype(NPBF)

        # q b-proj columns: 4 nope blocks then 2 rope pair blocks
        cols = [w_qb3[:, h, :DN] for h in heads]
        for pair in range(2):
            h0 = HL * d + 2 * pair
            cols.append(w_qb3[:, h0, DN:][:, _ROPE_PERM])
            cols.append(w_qb3[:, h0 + 1, DN:][:, _ROPE_PERM])
        wqb_local = np.concatenate(cols, axis=1)  # [1536, 768]
        wqb_t = np.ascontiguousarray(
            wqb_local.reshape(FQ, 128, 6, 128).transpose(2, 0, 1, 3)
        ).astype(NPBF)

        wkn_local = np.concatenate(
            [w_kvb3[:, h, :DN] for h in heads], axis=1)  # [512, 512]
        wkn_t = np.ascontiguousarray(
            wkn_local.reshape(FKV, 128, HL, 128).transpose(2, 0, 1, 3)
        ).astype(NPBF)

        wv_local = np.concatenate(
            [w_kvb3[:, h, DN:] for h in heads], axis=1)  # [512, 512]
        wv_t = np.ascontiguousarray(
            wv_local.reshape(FKV, 128, HL * DV)).astype(NPBF)

        wo_local = np.ascontiguousarray(
            w_o2[:, OSL * d:OSL * (d + 1)].reshape(NH * DV // 128, 128, OSL)
        ).astype(NPBF)

        in_maps.append({
            "hsT": hsT_d,
            "wa": wa_t,
            "wqb": wqb_t,
            "wkn": wkn_t,
            "wv": wv_t,
            "wo": wo_local,
            "cosq": cosq128,
            "sgnsinq": sgnsinq128,
            "cosk": np.tile(cos[:, TL * d:TL * (d + 1)], (2, 1)),
            "sgnsink": np.concatenate(
                [-sin[:, TL * d:TL * (d + 1)], sin[:, TL * d:TL * (d + 1)]],
                axis=0),
        })
    return in_maps


_CACHED_NC = {}


def _get_module(n_rep=1, upto="D"):
    key = (n_rep, upto)
    if key not in _CACHED_NC:
        _CACHED_NC[key] = build_module(n_rep, upto)
    return _CACHED_NC[key]


def run(in_maps, n_rep=1, upto="D", **kwargs):
    from concourse.bass_utils import run_bass_kernel_spmd
    nc = _get_module(n_rep, upto)
    return run_bass_kernel_spmd(nc, in_maps, core_ids=list(range(NCORES)),
                                **kwargs)


def kernel(**inputs):
    in_maps = _prepare_inputs(**inputs)
    res = run(in_maps)
    out = np.concatenate([res.results[d]["out_o"] for d in range(NCORES)],
                         axis=1)
    return out.astype(np.float32)

